# revision 21
# baseline (speedup 1.0000x reference)
"""Trainium2 Bass kernel for a 6-layer transformer decoder (self+cross attn + FFN).

Sharding: pure data-parallel over batch. B=8 sequences -> 8 NeuronCores,
one sequence per core; weights replicated. No collectives needed.

Device-side design (per core, one [S, D] sequence):
  - All matmuls in bf16 (fp32 PSUM accumulate); softmax/LN statistics fp32.
  - Activations kept in "normal" [s, d] layout for residual/LN (free-dim
    reductions), transposed to [d, s] via PE-transpose where a matmul needs
    the contraction dim on partitions.
  - Projections computed directly transposed (QT[d',s] = wq.T @ x.T with
    lhsT = wq) so attention logits can be built in [k, q] layout:
    LT[k,q] = KT.T-slice @ QT-slice.  Softmax runs over the partition (k)
    axis: exp on ScalarE, denominators come for free from a ones-column
    appended to V (AV matmul row 64 = sum_k exp), normalization via
    reciprocal + DRAM-round-trip partition-broadcast + elementwise multiply.
  - Causal masking is structural: upper-triangle blocks of LT are never
    computed; diagonal blocks are masked multiplicatively after exp.
"""

import math
from contextlib import ExitStack

import numpy as np
import ml_dtypes

import concourse.bass as bass
import concourse.bacc as bacc
import concourse.tile as tile
from concourse import mybir
from concourse.bass_utils import run_bass_kernel_spmd
from concourse.masks import make_identity, make_upper_triangular

# Problem dims (hardcoded per the harness contract).
L, D, H, F, V, MAXPOS = 6, 512, 8, 2048, 32000, 2048
B, S, SE = 8, 512, 512
DEP = D // H          # 64
P = 128
NST = S // P          # 4 sequence tiles
NDT = D // P          # 4 model-dim tiles
NFT = F // P          # 16 ffn-dim tiles
EPS = 1e-6
SQRT_D = math.sqrt(float(D))
INV_SQRT_DEP = 1.0 / math.sqrt(float(DEP))

BF16 = mybir.dt.bfloat16
F32 = mybir.dt.float32
F32R = mybir.dt.float32r
AF = mybir.ActivationFunctionType
OP = mybir.AluOpType

MASK_ZERO, MASK_CAUSAL, MASK_GENERIC = 0, 1, 2


def _bcast_mid(ap, n):
    """[P, X] AP -> [P, n, X] broadcast view (free-dim step-0 middle dim)."""
    return bass.AP(tensor=ap.tensor, offset=ap.offset, ap=[ap.ap[0], [0, n], *ap.ap[1:]])


def _bcast_part(ap, p):
    """1-D (or row) AP -> [p, ...] partition-broadcast view (step-0 partition).

    Only legal as a DMA source."""
    rest = [d for d in ap.ap if d[1] != 1] or [[1, 1]]
    return bass.AP(tensor=ap.tensor, offset=ap.offset, ap=[[0, p], *rest])


def _build_program(flags):
    (mask_mode, pm_on, bv_on, bo_on, b2_on, lnaff_on, b1_on) = flags

    nc = bacc.Bacc("TRN2", target_bir_lowering=False, debug=False, num_devices=B)

    tok_d = nc.declare_dram_parameter("tokens", [S], mybir.dt.int32, isOutput=False)
    enc_d = nc.declare_dram_parameter("enc", [SE, D], F32, isOutput=False)
    emb_d = nc.declare_dram_parameter("emb", [V, D], F32, isOutput=False)
    pe_d = nc.declare_dram_parameter("pe", [S, D], F32, isOutput=False)

    wnames = {}
    for prefix in ("self", "cross"):
        for nm in ("wq", "wk", "wv", "wo"):
            wnames[f"{prefix}_{nm}"] = nc.declare_dram_parameter(f"{prefix}_{nm}", [L, D, D], BF16, isOutput=False)
    w1_d = nc.declare_dram_parameter("ffn_w1", [L, D, F], BF16, isOutput=False)
    w2_d = nc.declare_dram_parameter("ffn_w2", [L, F, D], BF16, isOutput=False)

    bq_s_d = nc.declare_dram_parameter("self_bq", [L, D], F32, isOutput=False)
    bk_s_d = nc.declare_dram_parameter("self_bk", [L, D], F32, isOutput=False)
    bq_c_d = nc.declare_dram_parameter("cross_bq", [L, D], F32, isOutput=False)
    bk_c_d = nc.declare_dram_parameter("cross_bk", [L, D], F32, isOutput=False)
    b1_d = None
    if b1_on:
        b1_d = nc.declare_dram_parameter("ffn_b1", [L, F], F32, isOutput=False)
    bv_s_d = bv_c_d = bo_s_d = bo_c_d = b2_d = None
    if bv_on:
        bv_s_d = nc.declare_dram_parameter("self_bv", [L, D], F32, isOutput=False)
        bv_c_d = nc.declare_dram_parameter("cross_bv", [L, D], F32, isOutput=False)
    if bo_on:
        bo_s_d = nc.declare_dram_parameter("self_bo", [L, D], F32, isOutput=False)
        bo_c_d = nc.declare_dram_parameter("cross_bo", [L, D], F32, isOutput=False)
    if b2_on:
        b2_d = nc.declare_dram_parameter("ffn_b2", [L, D], F32, isOutput=False)
    ln_d = {}
    if lnaff_on:
        for i in (1, 2, 3):
            ln_d[f"g{i}"] = nc.declare_dram_parameter(f"ln{i}_g", [L, D], F32, isOutput=False)
            ln_d[f"b{i}"] = nc.declare_dram_parameter(f"ln{i}_b", [L, D], F32, isOutput=False)
    pm_d = None
    if pm_on:
        pm_d = nc.declare_dram_parameter("pm", [SE], F32, isOutput=False)  # already * -1e9
    maskT_d = None
    if mask_mode == MASK_GENERIC:
        maskT_d = nc.declare_dram_parameter("maskT", [S, S], F32, isOutput=False)  # [k, q] * -1e9

    out_d = nc.declare_dram_parameter("out", [S, D], F32, isOutput=True)

    with tile.TileContext(nc) as tc, ExitStack() as ctx:
        const = ctx.enter_context(tc.tile_pool(name="const", bufs=1))
        wpool = ctx.enter_context(tc.tile_pool(name="wpool", bufs=2))
        wbig = ctx.enter_context(tc.tile_pool(name="wbig", bufs=1))   # ffn weights
        apool = ctx.enter_context(tc.tile_pool(name="apool", bufs=2))
        hpool = ctx.enter_context(tc.tile_pool(name="hpool", bufs=1))  # ffn hidden
        rpool = ctx.enter_context(tc.tile_pool(name="rpool", bufs=3))  # residuals
        epool = ctx.enter_context(tc.tile_pool(name="epool", bufs=8))  # exp(LT)
        spool = ctx.enter_context(tc.tile_pool(name="spool", bufs=4))  # small stats
        dpool = ctx.enter_context(tc.tile_pool(name="dpool", bufs=2))  # denominators
        gpool = ctx.enter_context(tc.tile_pool(name="gpool", bufs=2))  # misc work
        dram = ctx.enter_context(tc.tile_pool(name="dram", bufs=2, space="DRAM"))
        pp = ctx.enter_context(tc.tile_pool(name="pp", bufs=2, space="PSUM"))
        # Per-h2 AV accumulators: [65, S] = one PSUM bank each, ring of 2.
        # PSUM budget: proj 2x1 + big 2x2 + av 2x1 = 8 banks (exactly full).
        ppo = ctx.enter_context(tc.tile_pool(name="ppo", bufs=2, space="PSUM"))

        ident = const.tile([P, P], BF16)
        make_identity(nc, ident[:])
        eps_t = const.tile([P, 1], F32)
        nc.vector.memset(eps_t[:], EPS)
        triu = None
        if mask_mode == MASK_CAUSAL:
            triu = const.tile([P, P], BF16)
            make_upper_triangular(nc, triu[:], val=1.0, diag=True)

        tok_sb = const.tile([P, NST], mybir.dt.int32)
        nc.sync.dma_start(out=tok_sb[:], in_=tok_d.rearrange("(t p) -> p t", p=P))
        pm_sb = None
        if pm_on:
            pm_sb = const.tile([P, NST], F32)
            nc.sync.dma_start(out=pm_sb[:], in_=pm_d.rearrange("(t p) -> p t", p=P))
        maskT_sb = None
        if mask_mode == MASK_GENERIC:
            maskT_sb = const.tile([P, NST, S], F32)
            nc.sync.dma_start(
                out=maskT_sb[:], in_=maskT_d.rearrange("(t p) q -> p t q", p=P)
            )

        # ---------- helpers ----------

        def load_w(d, l, kdim, ncols, tag, pool=None):
            t = (pool or wpool).tile([P, kdim, ncols], BF16, tag=tag)
            nc.sync.dma_start(out=t[:], in_=d[l].rearrange("(kt p) n -> p kt n", p=P))
            return t

        def load_bias_cols(d, l, n):
            t = wpool.tile([P, n], F32, tag="bias")
            nc.sync.dma_start(out=t[:, 0:n], in_=d[l].rearrange("(m p) -> p m", p=P))
            return t

        def load_bcast(d, l):
            t = wpool.tile([P, D], F32, tag="bcast")
            nc.gpsimd.dma_start(out=t[:], in_=_bcast_part(d[l], P))
            return t

        def transpose_into(src, dst, n_out_tiles=NDT, n_in_tiles=NST):
            """src [P, n_in_tiles, X] -> dst [P, n_out_tiles, Y] (PE transpose).

            Eviction on ScalarE (Copy): idle in transpose windows, frees DVE."""
            for dt in range(n_out_tiles):
                ps = pp.tile([P, n_in_tiles * P], BF16, tag="proj")
                for st in range(n_in_tiles):
                    nc.tensor.transpose(
                        ps[:, st * P:(st + 1) * P],
                        src[:, st, dt * P:(dt + 1) * P],
                        ident[:],
                    )
                nc.scalar.activation(dst[:, dt, :], ps[:], AF.Copy)

        def proj_T(w_sb, x_T, bias_sb, out_sb, engine="v"):
            """out_sb[d', s] = (x @ w + b).T ; lhsT = w, rhs = x.T.

            engine='s' evicts on ScalarE (for K projections, so the Q
            eviction on DVE and K eviction on ACT run concurrently)."""
            for m in range(NDT):
                ps = pp.tile([P, S], F32, tag="proj")
                for dt in range(NDT):
                    nc.tensor.matmul(
                        ps[:],
                        lhsT=w_sb[:, dt, m * P:(m + 1) * P],
                        rhs=x_T[:, dt, :],
                        start=(dt == 0),
                        stop=(dt == NDT - 1),
                    )
                if engine == "s":
                    bias = bias_sb[:, m:m + 1] if bias_sb is not None else 0.0
                    nc.scalar.activation(out_sb[:, m, :], ps[:], AF.Identity, bias=bias)
                elif bias_sb is not None:
                    nc.vector.tensor_scalar(
                        out_sb[:, m, :], ps[:], bias_sb[:, m:m + 1], None, OP.add
                    )
                else:
                    nc.vector.tensor_copy(out_sb[:, m, :], ps[:])

        def proj_V(w_sb, x_T, bias_bc, vg, nkt, engine="v"):
            """vg[k-tile][p, h*(DEP+1)+u] = V[k, 64h+u] (u<64), ones at u=64.

            engine='s' evicts on ScalarE -- only safe when ACT has no
            pending exp stream (i.e. the self-attn V, not the cross V)."""
            for st in range(nkt):
                ps = pp.tile([P, D], F32, tag="proj")
                for dt in range(NDT):
                    nc.tensor.matmul(
                        ps[:],
                        lhsT=x_T[:, dt, st * P:(st + 1) * P],
                        rhs=w_sb[:, dt, :],
                        start=(dt == 0),
                        stop=(dt == NDT - 1),
                    )
                src = ps[:].rearrange("p (h u) -> p h u", u=DEP)
                dstv = vg[:, st, :].rearrange("p (h u) -> p h u", u=DEP + 1)[:, :, 0:DEP]
                if bias_bc is not None:
                    nc.vector.tensor_tensor(
                        dstv, src, bias_bc[:].rearrange("p (h u) -> p h u", u=DEP), OP.add
                    )
                elif engine == "s":
                    nc.scalar.activation(dstv, src, AF.Copy)
                else:
                    nc.vector.tensor_copy(dstv, src)
            nc.vector.memset(
                vg[:].rearrange("p st (h u) -> p st h u", u=DEP + 1)[:, :, :, DEP:DEP + 1],
                1.0,
            )

        def attention(qt, kt_sb, vg, ot_sb, causal, use_pm):
            """Core attention: LT -> exp -> AV(+denominator) -> normalize.

            qt/kt_sb: [P, NDT, S] bf16 ([d', s]); vg: [P, NST, H*(DEP+1)];
            ot_sb out: [P, NDT, S] bf16 (attn output transposed [d', q]).

            AV accumulates per-h2 into 1-bank PSUM tiles (ring of 2) so the
            PE streams all 8 AV matmuls of a head-pair back-to-back while the
            previous tile drains on DVE.  The denominator (PSUM row DEP) is
            evicted raw, round-tripped through DRAM for the partition
            broadcast, and reciprocal'd only *after* the broadcast -- a full
            128-partition DVE op at partition offset 0 (reciprocal_approx_fast
            mis-executes at partition offsets != 0 on HW, and
            gpsimd.partition_broadcast is broken in this runtime).  The
            causal diagonal mask multiply runs on the otherwise-idle GpSimd."""
            def logits_exp(hp):
                explt = []
                for kt in range(NST):
                    qoff = kt * P if causal else 0
                    ps = pp.tile([P, 2, S], F32, tag="big")
                    for h2 in range(2):
                        nc.tensor.matmul(
                            ps[:, h2, qoff:],
                            lhsT=kt_sb[DEP * h2:DEP * (h2 + 1), hp, kt * P:(kt + 1) * P],
                            rhs=qt[DEP * h2:DEP * (h2 + 1), hp, qoff:],
                            start=True,
                            stop=True,
                        )
                    if maskT_sb is not None:
                        nc.vector.tensor_tensor(
                            ps[:], ps[:], _bcast_mid(maskT_sb[:, kt, :], 2), OP.add
                        )
                    et = epool.tile([P, 2, S], BF16, tag="explt")
                    bias = pm_sb[:, kt:kt + 1] if use_pm else 0.0
                    nc.scalar.activation(
                        et[:, :, qoff:], ps[:, :, qoff:], AF.Exp,
                        bias=bias, scale=INV_SQRT_DEP,
                    )
                    if causal:
                        # Diagonal-block causal mask on the otherwise-idle
                        # GpSimd: zero et where k_partition > q (iota = q - p).
                        dsl = slice(kt * P, (kt + 1) * P)
                        nc.gpsimd.affine_select(
                            out=et[:, :, dsl],
                            in_=et[:, :, dsl],
                            pattern=[[0, 2], [1, P]],
                            channel_multiplier=-1,
                            base=0,
                            compare_op=OP.is_ge,
                            fill=0.0,
                        )
                    explt.append(et)
                return explt

            def av_norm(hp, explt):
                den = dpool.tile([1, 2, S], F32, tag="den", bufs=4)
                scr = dram.tile([2, S], F32, tag="dscr", bufs=4)
                for h2 in range(2):
                    h = hp * 2 + h2
                    acc = ppo.tile([DEP + 1, S], F32, tag="av")
                    for kt in range(NST):
                        qoff = kt * P if causal else 0
                        nc.tensor.matmul(
                            acc[:, qoff:],
                            lhsT=vg[:, kt, h * (DEP + 1):(h + 1) * (DEP + 1)],
                            rhs=explt[kt][:, h2, qoff:],
                            start=(kt == 0),
                            stop=(kt == NST - 1),
                        )
                    nc.vector.tensor_copy(den[0:1, h2, :], acc[DEP:DEP + 1, :])
                    nc.sync.dma_start(out=scr[h2, :], in_=den[0:1, h2, :])
                    nc.vector.tensor_copy(
                        ot_sb[DEP * h2:DEP * (h2 + 1), hp, :], acc[0:DEP, :]
                    )
                rb = gpool.tile([P, S], F32, tag="recbc", bufs=4)
                for h2 in range(2):
                    nc.sync.dma_start(
                        out=rb[DEP * h2:DEP * (h2 + 1), :],
                        in_=_bcast_part(scr[h2, :], DEP),
                    )
                nc.vector.reciprocal_approx_fast(rb[:], rb[:])
                osl = ot_sb[:, hp, :]
                nc.vector.tensor_tensor(osl, osl, rb[:], OP.mult)

            # Software-pipelined by one head-pair: AV(hp-1) is emitted
            # between the exp streams of hp-1 and hp, so the PE FIFO
            # alternates logits/AV and the ACT exp stream never stalls
            # behind a block of AV matmuls (epool's 8 bufs hold exactly
            # two head-pairs' exp tiles).
            prev = None
            for hp in range(H // 2):
                explt = logits_exp(hp)
                if prev is not None:
                    av_norm(prev[0], prev[1])
                prev = (hp, explt)
            av_norm(prev[0], prev[1])

        def proj_O(w_sb, ot_sb, res_in, y_out, bo_bc):
            """y = attn_out @ wo (+bo) + res ; lhsT = ot_sb slices, rhs = wo."""
            for sm in range(NST):
                ps = pp.tile([P, D], F32, tag="proj")
                for dt in range(NDT):
                    nc.tensor.matmul(
                        ps[:],
                        lhsT=ot_sb[:, dt, sm * P:(sm + 1) * P],
                        rhs=w_sb[:, dt, :],
                        start=(dt == 0),
                        stop=(dt == NDT - 1),
                    )
                if bo_bc is not None:
                    nc.vector.tensor_tensor(ps[:], ps[:], bo_bc[:], OP.add)
                nc.vector.tensor_tensor(y_out[:, sm, :], ps[:], res_in[:, sm, :], OP.add)

        def layer_norm(y, out_res, g_bc, b_bc):
            # Fully per-s-tile so downstream consumers of tile st unblock as
            # soon as tile st's stats are done (no cross-tile barrier -- the
            # barrier costs more PE-idle/HAM-rethrottle than the extra tiny
            # ACT ops cost, since ACT is idle in LN windows anyway).
            for st in range(NST):
                st6 = spool.tile([P, 6], F32, tag="bst")
                nc.vector.bn_stats(st6[:], y[:, st, :])
                mv = spool.tile([P, 2], F32, tag="mv")
                nc.vector.bn_aggr(mv[:], st6[:])
                lnv = spool.tile([P, 1], F32, tag="lnv")
                nc.scalar.activation(lnv[:], mv[:, 1:2], AF.Ln, bias=eps_t[:, 0:1])
                rstd = spool.tile([P, 1], F32, tag="rstd")
                nc.scalar.activation(rstd[:], lnv[:], AF.Exp, scale=-0.5)
                nc.vector.tensor_scalar(
                    out_res[:, st, :], y[:, st, :],
                    mv[:, 0:1], rstd[:],
                    OP.subtract, OP.mult,
                )
                if g_bc is not None:
                    nc.vector.tensor_tensor(
                        out_res[:, st, :], out_res[:, st, :], g_bc[:], OP.mult
                    )
                if b_bc is not None:
                    nc.vector.tensor_tensor(
                        out_res[:, st, :], out_res[:, st, :], b_bc[:], OP.add
                    )

        def layer_norm_final(y, g_bc, b_bc):
            """Final LN: per s-tile f32 output tiles, DMA'd straight to out."""
            for st in range(NST):
                st6 = spool.tile([P, 6], F32, tag="bst")
                nc.vector.bn_stats(st6[:], y[:, st, :])
                mv = spool.tile([P, 2], F32, tag="mv")
                nc.vector.bn_aggr(mv[:], st6[:])
                lnv = spool.tile([P, 1], F32, tag="lnv")
                nc.scalar.activation(lnv[:], mv[:, 1:2], AF.Ln, bias=eps_t[:, 0:1])
                rstd = spool.tile([P, 1], F32, tag="rstd")
                nc.scalar.activation(rstd[:], lnv[:], AF.Exp, scale=-0.5)
                ot = gpool.tile([P, D], F32, tag="gather")
                nc.vector.tensor_scalar(
                    ot[:], y[:, st, :],
                    mv[:, 0:1], rstd[:],
                    OP.subtract, OP.mult,
                )
                if g_bc is not None:
                    nc.vector.tensor_tensor(ot[:], ot[:], g_bc[:], OP.mult)
                if b_bc is not None:
                    nc.vector.tensor_tensor(ot[:], ot[:], b_bc[:], OP.add)
                nc.sync.dma_start(out=out_d[st * P:(st + 1) * P, :], in_=ot[:])

        # ---------- program ----------

        # Embedding + positional encoding (fused scale+add on DVE; dedicated
        # tile tags so all four gathers issue without ring stalls).
        x = rpool.tile([P, NST, D], BF16, tag="res")
        for st in range(NST):
            g = gpool.tile([P, D], F32, tag="emb", bufs=4)
            nc.gpsimd.indirect_dma_start(
                out=g[:],
                out_offset=None,
                in_=emb_d[:, :],
                in_offset=bass.IndirectOffsetOnAxis(ap=tok_sb[:, st:st + 1], axis=0),
            )
            pe_t = gpool.tile([P, D], F32, tag="pet", bufs=4)
            nc.sync.dma_start(out=pe_t[:], in_=pe_d[st * P:(st + 1) * P, :])
            nc.vector.scalar_tensor_tensor(
                x[:, st, :], g[:], SQRT_D, pe_t[:], OP.mult, OP.add
            )

        encT = const.tile([P, NDT, SE], BF16)

        def cross_k(l):
            cwk = load_w(wnames["cross_wk"], l, NDT, D, "wqkv")
            bkc = load_bias_cols(bk_c_d, l, NDT)
            ktc = apool.tile([P, NDT, SE], BF16, tag="ktc")
            proj_T(cwk, encT, bkc, ktc)
            return ktc

        def cross_v(l):
            cwv = load_w(wnames["cross_wv"], l, NDT, D, "wqkv")
            bvc_bc = load_bcast(bv_c_d, l) if bv_on else None
            vgc = apool.tile([P, NST, H * (DEP + 1)], BF16, tag="vaugc")
            proj_V(cwv, encT, bvc_bc, vgc, SE // P)
            return vgc

        self_causal = mask_mode == MASK_CAUSAL

        for l in range(L):
            # ---- self attention ----
            xT = apool.tile([P, NDT, S], BF16, tag="rT")
            transpose_into(x, xT)
            swq = load_w(wnames["self_wq"], l, NDT, D, "wqkv")
            bq = load_bias_cols(bq_s_d, l, NDT)
            qt = apool.tile([P, NDT, S], BF16, tag="qt")
            proj_T(swq, xT, bq, qt)
            swk = load_w(wnames["self_wk"], l, NDT, D, "wqkv")
            bk = load_bias_cols(bk_s_d, l, NDT)
            kt_sb = apool.tile([P, NDT, S], BF16, tag="kt")
            proj_T(swk, xT, bk, kt_sb, engine="s")
            swv = load_w(wnames["self_wv"], l, NDT, D, "wqkv")
            bv_bc = load_bcast(bv_s_d, l) if bv_on else None
            vg = apool.tile([P, NST, H * (DEP + 1)], BF16, tag="vaug")
            proj_V(swv, xT, bv_bc, vg, NST, engine="s")
            if l == 0:
                # Encoder output, transposed once (shared across layers).
                # Emitted after layer-0 QKV so the startup critical path
                # (embed -> xT -> q/k/v) owns the PE first; first needed
                # below for the cross-attn K projection.
                enc_bf = apool.tile([P, NST, D], BF16, tag="y")
                for st in range(NST):
                    et = gpool.tile([P, D], F32, tag="emb", bufs=4)
                    nc.sync.dma_start(out=et[:], in_=enc_d[st * P:(st + 1) * P, :])
                    nc.vector.tensor_copy(enc_bf[:, st, :], et[:])
                transpose_into(enc_bf, encT)
            ot_sb = apool.tile([P, NDT, S], BF16, tag="ot")
            attention(qt, kt_sb, vg, ot_sb, self_causal, False)
            # Cross-attn K/V depend only on encT; one is emitted inside each
            # attention window as PE filler: V(l) here (fills attn1's exp
            # stream), K(l+1) after attention2 below (fills attn2's).
            ktc = cross_k(0) if l == 0 else ktc_next
            vgc = cross_v(l)
            swo = load_w(wnames["self_wo"], l, NDT, D, "wqkv")
            bo_bc = load_bcast(bo_s_d, l) if bo_on else None
            y1 = apool.tile([P, NST, D], BF16, tag="y")
            proj_O(swo, ot_sb, x, y1, bo_bc)
            out1 = rpool.tile([P, NST, D], BF16, tag="res")
            g_bc = load_bcast(ln_d["g1"], l) if lnaff_on else None
            b_bc = load_bcast(ln_d["b1"], l) if lnaff_on else None
            layer_norm(y1, out1, g_bc, b_bc)

            # ---- cross attention ----
            o1T = apool.tile([P, NDT, S], BF16, tag="rT")
            transpose_into(out1, o1T)
            cwq = load_w(wnames["cross_wq"], l, NDT, D, "wqkv")
            bqc = load_bias_cols(bq_c_d, l, NDT)
            qt2 = apool.tile([P, NDT, S], BF16, tag="qt")
            proj_T(cwq, o1T, bqc, qt2)
            ot2 = apool.tile([P, NDT, S], BF16, tag="ot")
            attention(qt2, ktc, vgc, ot2, False, pm_on)
            if l + 1 < L:
                # Next layer's cross K: PE filler for attn2's exp stream.
                ktc_next = cross_k(l + 1)
            cwo = load_w(wnames["cross_wo"], l, NDT, D, "wqkv")
            boc_bc = load_bcast(bo_c_d, l) if bo_on else None
            y2 = apool.tile([P, NST, D], BF16, tag="y")
            proj_O(cwo, ot2, out1, y2, boc_bc)
            out2 = rpool.tile([P, NST, D], BF16, tag="res")
            g_bc = load_bcast(ln_d["g2"], l) if lnaff_on else None
            b_bc = load_bcast(ln_d["b2"], l) if lnaff_on else None
            layer_norm(y2, out2, g_bc, b_bc)

            # ---- FFN ----
            o2T = apool.tile([P, NDT, S], BF16, tag="rT")
            transpose_into(out2, o2T)
            w1s = load_w(w1_d, l, NDT, F, "w1", pool=wbig)
            b1 = load_bias_cols(b1_d, l, NFT) if b1_on else None
            h_sb = hpool.tile([P, NFT, S], BF16, tag="hsb")
            for fm in range(NFT // 2):
                ps = pp.tile([P, 2, S], F32, tag="big")
                for f2 in range(2):
                    ft = fm * 2 + f2
                    for dt in range(NDT):
                        nc.tensor.matmul(
                            ps[:, f2, :],
                            lhsT=w1s[:, dt, ft * P:(ft + 1) * P],
                            rhs=o2T[:, dt, :],
                            start=(dt == 0),
                            stop=(dt == NDT - 1),
                        )
                if b1 is None:
                    nc.vector.tensor_scalar(
                        h_sb[:, fm * 2:fm * 2 + 2, :], ps[:], 0.0, None, OP.max
                    )
                else:
                    for f2 in range(2):
                        ft = fm * 2 + f2
                        nc.vector.tensor_scalar(
                            h_sb[:, ft, :], ps[:, f2, :],
                            b1[:, ft:ft + 1], 0.0, OP.add, OP.max,
                        )
            w2s = load_w(w2_d, l, NFT, D, "w2", pool=wbig)
            b2_bc = load_bcast(b2_d, l) if b2_on else None
            y3 = apool.tile([P, NST, D], BF16, tag="y")
            for sm in range(NST):
                ps2 = pp.tile([P, D], F32, tag="proj")
                for ft in range(NFT):
                    nc.tensor.matmul(
                        ps2[:],
                        lhsT=h_sb[:, ft, sm * P:(sm + 1) * P],
                        rhs=w2s[:, ft, :],
                        start=(ft == 0),
                        stop=(ft == NFT - 1),
                    )
                if b2_bc is not None:
                    nc.vector.tensor_tensor(ps2[:], ps2[:], b2_bc[:], OP.add)
                nc.vector.tensor_tensor(y3[:, sm, :], ps2[:], out2[:, sm, :], OP.add)
            g_bc = load_bcast(ln_d["g3"], l) if lnaff_on else None
            b_bc = load_bcast(ln_d["b3"], l) if lnaff_on else None
            if l == L - 1:
                layer_norm_final(y3, g_bc, b_bc)
            else:
                x = rpool.tile([P, NST, D], BF16, tag="res")
                layer_norm(y3, x, g_bc, b_bc)

    # All ScalarE functions we use (Exp, Ln, Identity, Relu, Copy) coexist in
    # the "natural_log_exp_and_others" table set.  Left to itself, the
    # table-load pass resolves Exp and Ln to *different* sets and thrashes
    # ACT_TABLE_LOADs (~1.3us each, hundreds of times).  Strip our functions
    # from every other set (keeping set order/ids intact for walrus) so all
    # activations resolve to the one shared set -> a single table load.
    _keep = "natural_log_exp_and_others"
    _mine = {AF.Exp, AF.Ln, AF.Identity, AF.Relu, AF.Copy}
    _orig = bacc.get_activation_tables

    def _pinned_tables(arch):
        tabs = _orig(arch)
        return {
            name: (fns if name == _keep else fns - _mine)
            for name, fns in tabs.items()
        }

    bacc.get_activation_tables = _pinned_tables
    try:
        nc.finalize()
    finally:
        bacc.get_activation_tables = _orig
    return nc


_PROGRAM_CACHE = {}


def _get_program(flags):
    if flags not in _PROGRAM_CACHE:
        _PROGRAM_CACHE[flags] = _build_program(flags)
    return _PROGRAM_CACHE[flags]


def _positional_encoding(max_pos, d_model):
    pos = np.arange(max_pos, dtype=np.float32)[:, None]
    i = np.arange(d_model, dtype=np.float32)[None, :]
    angle = pos / np.power(10000.0, 2.0 * np.floor(i / 2.0) / d_model)
    pe = np.where(
        np.arange(d_model)[None, :] % 2 == 0,
        np.sin(angle, dtype=np.float32),
        np.cos(angle, dtype=np.float32),
    )
    return pe.astype(np.float32)


def _classify_mask(la):
    la2 = np.asarray(la).reshape(S, S)
    if not la2.any():
        return MASK_ZERO
    causal = np.triu(np.ones((S, S), np.float32), 1)
    if np.array_equal(la2.astype(np.float32), causal):
        return MASK_CAUSAL
    return MASK_GENERIC


def kernel(**inputs):
    f32 = np.float32
    tokens = np.asarray(inputs["tokens"]).astype(np.int32)
    enc = np.asarray(inputs["enc_output"]).astype(f32)
    la = np.asarray(inputs["look_ahead_mask"]).astype(f32)
    pad = np.asarray(inputs["padding_mask"]).astype(f32)
    emb = np.asarray(inputs["emb"]).astype(f32)

    mask_mode = _classify_mask(la)
    pm_on = bool(pad.any())
    bv_on = bool(
        np.asarray(inputs["self_bv"]).any() or np.asarray(inputs["cross_bv"]).any()
    )
    bo_on = bool(
        np.asarray(inputs["self_bo"]).any() or np.asarray(inputs["cross_bo"]).any()
    )
    b2_on = bool(np.asarray(inputs["ffn_b2"]).any())
    b1_on = bool(np.asarray(inputs["ffn_b1"]).any())
    lnaff_on = any(
        not np.allclose(np.asarray(inputs[f"ln{i}_g"]), 1.0)
        or np.asarray(inputs[f"ln{i}_b"]).any()
        for i in (1, 2, 3)
    )
    flags = (mask_mode, pm_on, bv_on, bo_on, b2_on, lnaff_on, b1_on)
    nc = _get_program(flags)

    bf16 = ml_dtypes.bfloat16
    common = {
        "emb": emb,
        "pe": _positional_encoding(MAXPOS, D)[:S],
        "ffn_w1": np.asarray(inputs["ffn_w1"]).astype(bf16),
        "ffn_w2": np.asarray(inputs["ffn_w2"]).astype(bf16),
        "self_bq": np.asarray(inputs["self_bq"]).astype(f32),
        "self_bk": np.asarray(inputs["self_bk"]).astype(f32),
        "cross_bq": np.asarray(inputs["cross_bq"]).astype(f32),
        "cross_bk": np.asarray(inputs["cross_bk"]).astype(f32),
    }
    if b1_on:
        common["ffn_b1"] = np.asarray(inputs["ffn_b1"]).astype(f32)
    for prefix in ("self", "cross"):
        for nm in ("wq", "wk", "wv", "wo"):
            common[f"{prefix}_{nm}"] = np.asarray(inputs[f"{prefix}_{nm}"]).astype(bf16)
    if bv_on:
        common["self_bv"] = np.asarray(inputs["self_bv"]).astype(f32)
        common["cross_bv"] = np.asarray(inputs["cross_bv"]).astype(f32)
    if bo_on:
        common["self_bo"] = np.asarray(inputs["self_bo"]).astype(f32)
        common["cross_bo"] = np.asarray(inputs["cross_bo"]).astype(f32)
    if b2_on:
        common["ffn_b2"] = np.asarray(inputs["ffn_b2"]).astype(f32)
    if lnaff_on:
        for i in (1, 2, 3):
            common[f"ln{i}_g"] = np.asarray(inputs[f"ln{i}_g"]).astype(f32)
            common[f"ln{i}_b"] = np.asarray(inputs[f"ln{i}_b"]).astype(f32)
    if mask_mode == MASK_GENERIC:
        common["maskT"] = np.ascontiguousarray(
            la.reshape(S, S).T * np.float32(-1e9)
        ).astype(f32)

    in_maps = []
    for b in range(B):
        m = dict(common)
        m["tokens"] = np.ascontiguousarray(tokens[b])
        m["enc"] = np.ascontiguousarray(enc[b])
        if pm_on:
            m["pm"] = np.ascontiguousarray(pad[b, 0, 0] * np.float32(-1e9))
        in_maps.append(m)

    global _last_flags, _last_in_maps
    _last_flags = flags
    _last_in_maps = in_maps

    res = run_bass_kernel_spmd(nc, in_maps, list(range(B)))
    out = np.stack([res.results[i]["out"] for i in range(B)], axis=0)
    return out.astype(np.float32)


_last_flags = None
_last_in_maps = None



# revision 26
# speedup vs baseline: 1.0074x; 1.0074x over previous
"""Trainium2 Bass kernel for a 6-layer transformer decoder (self+cross attn + FFN).

Sharding: pure data-parallel over batch. B=8 sequences -> 8 NeuronCores,
one sequence per core; weights replicated. No collectives needed.

Device-side design (per core, one [S, D] sequence):
  - All matmuls in bf16 (fp32 PSUM accumulate); softmax/LN statistics fp32.
  - Activations kept in "normal" [s, d] layout for residual/LN (free-dim
    reductions), transposed to [d, s] via PE-transpose where a matmul needs
    the contraction dim on partitions.
  - Projections computed directly transposed (QT[d',s] = wq.T @ x.T with
    lhsT = wq) so attention logits can be built in [k, q] layout:
    LT[k,q] = KT.T-slice @ QT-slice.  Softmax runs over the partition (k)
    axis: exp on ScalarE, denominators come for free from a ones-column
    appended to V (AV matmul row 64 = sum_k exp), normalization via
    reciprocal + DRAM-round-trip partition-broadcast + elementwise multiply.
  - Causal masking is structural: upper-triangle blocks of LT are never
    computed; diagonal blocks are masked multiplicatively after exp.
"""

import math
from contextlib import ExitStack

import numpy as np
import ml_dtypes

import concourse.bass as bass
import concourse.bacc as bacc
import concourse.tile as tile
from concourse import mybir
from concourse.bass_utils import run_bass_kernel_spmd
from concourse.masks import make_identity, make_upper_triangular

# Problem dims (hardcoded per the harness contract).
L, D, H, F, V, MAXPOS = 6, 512, 8, 2048, 32000, 2048
B, S, SE = 8, 512, 512
DEP = D // H          # 64
P = 128
NST = S // P          # 4 sequence tiles
NDT = D // P          # 4 model-dim tiles
NFT = F // P          # 16 ffn-dim tiles
EPS = 1e-6
SQRT_D = math.sqrt(float(D))
INV_SQRT_DEP = 1.0 / math.sqrt(float(DEP))

BF16 = mybir.dt.bfloat16
F32 = mybir.dt.float32
F32R = mybir.dt.float32r
AF = mybir.ActivationFunctionType
OP = mybir.AluOpType

MASK_ZERO, MASK_CAUSAL, MASK_GENERIC = 0, 1, 2


def _bcast_mid(ap, n):
    """[P, X] AP -> [P, n, X] broadcast view (free-dim step-0 middle dim)."""
    return bass.AP(tensor=ap.tensor, offset=ap.offset, ap=[ap.ap[0], [0, n], *ap.ap[1:]])


def _bcast_part(ap, p):
    """1-D (or row) AP -> [p, ...] partition-broadcast view (step-0 partition).

    Only legal as a DMA source."""
    rest = [d for d in ap.ap if d[1] != 1] or [[1, 1]]
    return bass.AP(tensor=ap.tensor, offset=ap.offset, ap=[[0, p], *rest])


def _build_program(flags):
    (mask_mode, pm_on, bv_on, bo_on, b2_on, lnaff_on, b1_on) = flags

    nc = bacc.Bacc("TRN2", target_bir_lowering=False, debug=False, num_devices=B)

    tok_d = nc.declare_dram_parameter("tokens", [S], mybir.dt.int32, isOutput=False)
    enc_d = nc.declare_dram_parameter("enc", [SE, D], F32, isOutput=False)
    emb_d = nc.declare_dram_parameter("emb", [V, D], F32, isOutput=False)
    pe_d = nc.declare_dram_parameter("pe", [S, D], F32, isOutput=False)

    wnames = {}
    for prefix in ("self", "cross"):
        for nm in ("wq", "wk", "wv", "wo"):
            wnames[f"{prefix}_{nm}"] = nc.declare_dram_parameter(f"{prefix}_{nm}", [L, D, D], BF16, isOutput=False)
    w1_d = nc.declare_dram_parameter("ffn_w1", [L, D, F], BF16, isOutput=False)
    w2_d = nc.declare_dram_parameter("ffn_w2", [L, F, D], BF16, isOutput=False)

    bq_s_d = nc.declare_dram_parameter("self_bq", [L, D], F32, isOutput=False)
    bk_s_d = nc.declare_dram_parameter("self_bk", [L, D], F32, isOutput=False)
    bq_c_d = nc.declare_dram_parameter("cross_bq", [L, D], F32, isOutput=False)
    bk_c_d = nc.declare_dram_parameter("cross_bk", [L, D], F32, isOutput=False)
    b1_d = None
    if b1_on:
        b1_d = nc.declare_dram_parameter("ffn_b1", [L, F], F32, isOutput=False)
    bv_s_d = bv_c_d = bo_s_d = bo_c_d = b2_d = None
    if bv_on:
        bv_s_d = nc.declare_dram_parameter("self_bv", [L, D], F32, isOutput=False)
        bv_c_d = nc.declare_dram_parameter("cross_bv", [L, D], F32, isOutput=False)
    if bo_on:
        bo_s_d = nc.declare_dram_parameter("self_bo", [L, D], F32, isOutput=False)
        bo_c_d = nc.declare_dram_parameter("cross_bo", [L, D], F32, isOutput=False)
    if b2_on:
        b2_d = nc.declare_dram_parameter("ffn_b2", [L, D], F32, isOutput=False)
    ln_d = {}
    if lnaff_on:
        for i in (1, 2, 3):
            ln_d[f"g{i}"] = nc.declare_dram_parameter(f"ln{i}_g", [L, D], F32, isOutput=False)
            ln_d[f"b{i}"] = nc.declare_dram_parameter(f"ln{i}_b", [L, D], F32, isOutput=False)
    pm_d = None
    if pm_on:
        pm_d = nc.declare_dram_parameter("pm", [SE], F32, isOutput=False)  # already * -1e9
    maskT_d = None
    if mask_mode == MASK_GENERIC:
        maskT_d = nc.declare_dram_parameter("maskT", [S, S], F32, isOutput=False)  # [k, q] * -1e9

    out_d = nc.declare_dram_parameter("out", [S, D], F32, isOutput=True)

    with tile.TileContext(nc) as tc, ExitStack() as ctx:
        const = ctx.enter_context(tc.tile_pool(name="const", bufs=1))
        wpool = ctx.enter_context(tc.tile_pool(name="wpool", bufs=2))
        wbig = ctx.enter_context(tc.tile_pool(name="wbig", bufs=1))   # ffn weights
        apool = ctx.enter_context(tc.tile_pool(name="apool", bufs=2))
        hpool = ctx.enter_context(tc.tile_pool(name="hpool", bufs=1))  # ffn hidden
        rpool = ctx.enter_context(tc.tile_pool(name="rpool", bufs=3))  # residuals
        epool = ctx.enter_context(tc.tile_pool(name="epool", bufs=8))  # exp(LT)
        spool = ctx.enter_context(tc.tile_pool(name="spool", bufs=4))  # small stats
        dpool = ctx.enter_context(tc.tile_pool(name="dpool", bufs=2))  # denominators
        gpool = ctx.enter_context(tc.tile_pool(name="gpool", bufs=2))  # misc work
        dram = ctx.enter_context(tc.tile_pool(name="dram", bufs=2, space="DRAM"))
        pp = ctx.enter_context(tc.tile_pool(name="pp", bufs=2, space="PSUM"))
        # Per-h2 AV accumulators: [65, S] = one PSUM bank each, ring of 2.
        # PSUM budget: proj 2x1 + big 2x2 + av 2x1 = 8 banks (exactly full).
        ppo = ctx.enter_context(tc.tile_pool(name="ppo", bufs=2, space="PSUM"))

        ident = const.tile([P, P], BF16)
        make_identity(nc, ident[:])
        eps_t = const.tile([P, 1], F32)
        nc.vector.memset(eps_t[:], EPS)
        triu = None
        if mask_mode == MASK_CAUSAL:
            triu = const.tile([P, P], BF16)
            make_upper_triangular(nc, triu[:], val=1.0, diag=True)

        tok_sb = const.tile([P, NST], mybir.dt.int32)
        nc.sync.dma_start(out=tok_sb[:], in_=tok_d.rearrange("(t p) -> p t", p=P))
        pm_sb = None
        if pm_on:
            pm_sb = const.tile([P, NST], F32)
            nc.sync.dma_start(out=pm_sb[:], in_=pm_d.rearrange("(t p) -> p t", p=P))
        maskT_sb = None
        if mask_mode == MASK_GENERIC:
            maskT_sb = const.tile([P, NST, S], F32)
            nc.sync.dma_start(
                out=maskT_sb[:], in_=maskT_d.rearrange("(t p) q -> p t q", p=P)
            )

        # ---------- helpers ----------

        def load_w(d, l, kdim, ncols, tag, pool=None):
            t = (pool or wpool).tile([P, kdim, ncols], BF16, tag=tag)
            nc.sync.dma_start(out=t[:], in_=d[l].rearrange("(kt p) n -> p kt n", p=P))
            return t

        def load_bias_cols(d, l, n):
            t = wpool.tile([P, n], F32, tag="bias")
            nc.sync.dma_start(out=t[:, 0:n], in_=d[l].rearrange("(m p) -> p m", p=P))
            return t

        def load_bcast(d, l):
            t = wpool.tile([P, D], F32, tag="bcast")
            nc.gpsimd.dma_start(out=t[:], in_=_bcast_part(d[l], P))
            return t

        def transpose_into(src, dst, n_out_tiles=NDT, n_in_tiles=NST):
            """src [P, n_in_tiles, X] -> dst [P, n_out_tiles, Y] (PE transpose).

            Eviction on ScalarE (Copy): idle in transpose windows, frees DVE."""
            for dt in range(n_out_tiles):
                ps = pp.tile([P, n_in_tiles * P], BF16, tag="proj")
                for st in range(n_in_tiles):
                    nc.tensor.transpose(
                        ps[:, st * P:(st + 1) * P],
                        src[:, st, dt * P:(dt + 1) * P],
                        ident[:],
                    )
                nc.scalar.activation(dst[:, dt, :], ps[:], AF.Copy)

        def proj_T(w_sb, x_T, bias_sb, out_sb, engine="v"):
            """out_sb[d', s] = (x @ w + b).T ; lhsT = w, rhs = x.T.

            engine='s' evicts on ScalarE (for K projections, so the Q
            eviction on DVE and K eviction on ACT run concurrently)."""
            for m in range(NDT):
                ps = pp.tile([P, S], F32, tag="proj")
                for dt in range(NDT):
                    nc.tensor.matmul(
                        ps[:],
                        lhsT=w_sb[:, dt, m * P:(m + 1) * P],
                        rhs=x_T[:, dt, :],
                        start=(dt == 0),
                        stop=(dt == NDT - 1),
                    )
                if engine == "s":
                    bias = bias_sb[:, m:m + 1] if bias_sb is not None else 0.0
                    nc.scalar.activation(out_sb[:, m, :], ps[:], AF.Identity, bias=bias)
                elif bias_sb is not None:
                    nc.vector.tensor_scalar(
                        out_sb[:, m, :], ps[:], bias_sb[:, m:m + 1], None, OP.add
                    )
                else:
                    nc.vector.tensor_copy(out_sb[:, m, :], ps[:])

        def proj_V(w_sb, x_T, bias_bc, vg, nkt, engine="v"):
            """vg[k-tile][p, h*(DEP+1)+u] = V[k, 64h+u] (u<64), ones at u=64.

            engine='s' evicts on ScalarE -- only safe when ACT has no
            pending exp stream (i.e. the self-attn V, not the cross V)."""
            for st in range(nkt):
                ps = pp.tile([P, D], F32, tag="proj")
                for dt in range(NDT):
                    nc.tensor.matmul(
                        ps[:],
                        lhsT=x_T[:, dt, st * P:(st + 1) * P],
                        rhs=w_sb[:, dt, :],
                        start=(dt == 0),
                        stop=(dt == NDT - 1),
                    )
                src = ps[:].rearrange("p (h u) -> p h u", u=DEP)
                dstv = vg[:, st, :].rearrange("p (h u) -> p h u", u=DEP + 1)[:, :, 0:DEP]
                if bias_bc is not None:
                    nc.vector.tensor_tensor(
                        dstv, src, bias_bc[:].rearrange("p (h u) -> p h u", u=DEP), OP.add
                    )
                elif engine == "s":
                    nc.scalar.activation(dstv, src, AF.Copy)
                else:
                    nc.vector.tensor_copy(dstv, src)
            nc.vector.memset(
                vg[:].rearrange("p st (h u) -> p st h u", u=DEP + 1)[:, :, :, DEP:DEP + 1],
                1.0,
            )

        def attention(qt, kt_sb, vg, ot_sb, causal, use_pm):
            """Core attention: LT -> exp -> AV(+denominator) -> normalize.

            qt/kt_sb: [P, NDT, S] bf16 ([d', s]); vg: [P, NST, H*(DEP+1)];
            ot_sb out: [P, NDT, S] bf16 (attn output transposed [d', q]).

            AV accumulates per-h2 into 1-bank PSUM tiles (ring of 2) so the
            PE streams all 8 AV matmuls of a head-pair back-to-back while the
            previous tile drains on DVE.  The denominator (PSUM row DEP) is
            evicted raw, round-tripped through DRAM for the partition
            broadcast, and reciprocal'd only *after* the broadcast -- a full
            128-partition DVE op at partition offset 0 (reciprocal_approx_fast
            mis-executes at partition offsets != 0 on HW, and
            gpsimd.partition_broadcast is broken in this runtime).  The
            causal diagonal mask multiply runs on the otherwise-idle GpSimd."""
            def logits_exp(hp):
                explt = []
                for kt in range(NST):
                    qoff = kt * P if causal else 0
                    ps = pp.tile([P, 2, S], F32, tag="big")
                    for h2 in range(2):
                        nc.tensor.matmul(
                            ps[:, h2, qoff:],
                            lhsT=kt_sb[DEP * h2:DEP * (h2 + 1), hp, kt * P:(kt + 1) * P],
                            rhs=qt[DEP * h2:DEP * (h2 + 1), hp, qoff:],
                            start=True,
                            stop=True,
                        )
                    if maskT_sb is not None:
                        nc.vector.tensor_tensor(
                            ps[:], ps[:], _bcast_mid(maskT_sb[:, kt, :], 2), OP.add
                        )
                    et = epool.tile([P, 2, S], BF16, tag="explt")
                    bias = pm_sb[:, kt:kt + 1] if use_pm else 0.0
                    nc.scalar.activation(
                        et[:, :, qoff:], ps[:, :, qoff:], AF.Exp,
                        bias=bias, scale=INV_SQRT_DEP,
                    )
                    if causal:
                        # Diagonal-block causal mask on the otherwise-idle
                        # GpSimd: zero et where k_partition > q (iota = q - p).
                        dsl = slice(kt * P, (kt + 1) * P)
                        nc.gpsimd.affine_select(
                            out=et[:, :, dsl],
                            in_=et[:, :, dsl],
                            pattern=[[0, 2], [1, P]],
                            channel_multiplier=-1,
                            base=0,
                            compare_op=OP.is_ge,
                            fill=0.0,
                        )
                    explt.append(et)
                return explt

            def av_norm(hp, explt):
                den = dpool.tile([1, 2, S], F32, tag="den", bufs=4)
                scr = dram.tile([2, S], F32, tag="dscr", bufs=4)
                for h2 in range(2):
                    h = hp * 2 + h2
                    acc = ppo.tile([DEP + 1, S], F32, tag="av")
                    for kt in range(NST):
                        qoff = kt * P if causal else 0
                        nc.tensor.matmul(
                            acc[:, qoff:],
                            lhsT=vg[:, kt, h * (DEP + 1):(h + 1) * (DEP + 1)],
                            rhs=explt[kt][:, h2, qoff:],
                            start=(kt == 0),
                            stop=(kt == NST - 1),
                        )
                    nc.vector.tensor_copy(den[0:1, h2, :], acc[DEP:DEP + 1, :])
                    nc.sync.dma_start(out=scr[h2, :], in_=den[0:1, h2, :])
                    nc.vector.tensor_copy(
                        ot_sb[DEP * h2:DEP * (h2 + 1), hp, :], acc[0:DEP, :]
                    )
                rb = gpool.tile([P, S], F32, tag="recbc", bufs=4)
                for h2 in range(2):
                    nc.sync.dma_start(
                        out=rb[DEP * h2:DEP * (h2 + 1), :],
                        in_=_bcast_part(scr[h2, :], DEP),
                    )
                nc.vector.reciprocal_approx_fast(rb[:], rb[:])
                osl = ot_sb[:, hp, :]
                nc.vector.tensor_tensor(osl, osl, rb[:], OP.mult)

            for hp in range(H // 2):
                av_norm(hp, logits_exp(hp))

        def proj_O(w_sb, ot_sb, res_in, y_out, bo_bc):
            """y = attn_out @ wo (+bo) + res ; lhsT = ot_sb slices, rhs = wo."""
            for sm in range(NST):
                ps = pp.tile([P, D], F32, tag="proj")
                for dt in range(NDT):
                    nc.tensor.matmul(
                        ps[:],
                        lhsT=ot_sb[:, dt, sm * P:(sm + 1) * P],
                        rhs=w_sb[:, dt, :],
                        start=(dt == 0),
                        stop=(dt == NDT - 1),
                    )
                if bo_bc is not None:
                    nc.vector.tensor_tensor(ps[:], ps[:], bo_bc[:], OP.add)
                nc.vector.tensor_tensor(y_out[:, sm, :], ps[:], res_in[:, sm, :], OP.add)

        def _ln_stats(y):
            """Batched LN stats: one Ln and one Exp over all NST tiles
            (amortizes the ~352-cycle per-ACTIVATE overhead 4x)."""
            mv = spool.tile([P, NST, 2], F32, tag="mv")
            for st in range(NST):
                st6 = spool.tile([P, 6], F32, tag="bst")
                nc.vector.bn_stats(st6[:], y[:, st, :])
                nc.vector.bn_aggr(mv[:, st, :], st6[:])
            lnv = spool.tile([P, NST, 1], F32, tag="lnv")
            nc.scalar.activation(lnv[:], mv[:, :, 1:2], AF.Ln, bias=eps_t[:, 0:1])
            rstd = spool.tile([P, NST, 1], F32, tag="rstd")
            nc.scalar.activation(rstd[:], lnv[:], AF.Exp, scale=-0.5)
            return mv, rstd

        def layer_norm(y, out_res, g_bc, b_bc):
            mv, rstd = _ln_stats(y)
            for st in range(NST):
                nc.vector.tensor_scalar(
                    out_res[:, st, :], y[:, st, :],
                    mv[:, st, 0:1], rstd[:, st, :],
                    OP.subtract, OP.mult,
                )
                if g_bc is not None:
                    nc.vector.tensor_tensor(
                        out_res[:, st, :], out_res[:, st, :], g_bc[:], OP.mult
                    )
                if b_bc is not None:
                    nc.vector.tensor_tensor(
                        out_res[:, st, :], out_res[:, st, :], b_bc[:], OP.add
                    )

        def layer_norm_final(y, g_bc, b_bc):
            """Final LN: per s-tile f32 output tiles, DMA'd straight to out."""
            mv, rstd = _ln_stats(y)
            for st in range(NST):
                ot = gpool.tile([P, D], F32, tag="gather")
                nc.vector.tensor_scalar(
                    ot[:], y[:, st, :],
                    mv[:, st, 0:1], rstd[:, st, :],
                    OP.subtract, OP.mult,
                )
                if g_bc is not None:
                    nc.vector.tensor_tensor(ot[:], ot[:], g_bc[:], OP.mult)
                if b_bc is not None:
                    nc.vector.tensor_tensor(ot[:], ot[:], b_bc[:], OP.add)
                nc.sync.dma_start(out=out_d[st * P:(st + 1) * P, :], in_=ot[:])

        # ---------- program ----------

        # Embedding + positional encoding (fused scale+add on DVE; dedicated
        # tile tags so all four gathers issue without ring stalls).
        x = rpool.tile([P, NST, D], BF16, tag="res")
        for st in range(NST):
            g = gpool.tile([P, D], F32, tag="emb", bufs=4)
            nc.gpsimd.indirect_dma_start(
                out=g[:],
                out_offset=None,
                in_=emb_d[:, :],
                in_offset=bass.IndirectOffsetOnAxis(ap=tok_sb[:, st:st + 1], axis=0),
            )
            pe_t = gpool.tile([P, D], F32, tag="pet", bufs=4)
            nc.sync.dma_start(out=pe_t[:], in_=pe_d[st * P:(st + 1) * P, :])
            nc.vector.scalar_tensor_tensor(
                x[:, st, :], g[:], SQRT_D, pe_t[:], OP.mult, OP.add
            )

        encT = const.tile([P, NDT, SE], BF16)

        def cross_k(l):
            cwk = load_w(wnames["cross_wk"], l, NDT, D, "wqkv")
            bkc = load_bias_cols(bk_c_d, l, NDT)
            ktc = apool.tile([P, NDT, SE], BF16, tag="ktc")
            proj_T(cwk, encT, bkc, ktc)
            return ktc

        def cross_v(l):
            cwv = load_w(wnames["cross_wv"], l, NDT, D, "wqkv")
            bvc_bc = load_bcast(bv_c_d, l) if bv_on else None
            vgc = apool.tile([P, NST, H * (DEP + 1)], BF16, tag="vaugc")
            proj_V(cwv, encT, bvc_bc, vgc, SE // P)
            return vgc

        self_causal = mask_mode == MASK_CAUSAL

        for l in range(L):
            # ---- self attention ----
            xT = apool.tile([P, NDT, S], BF16, tag="rT")
            transpose_into(x, xT)
            swq = load_w(wnames["self_wq"], l, NDT, D, "wqkv")
            bq = load_bias_cols(bq_s_d, l, NDT)
            qt = apool.tile([P, NDT, S], BF16, tag="qt")
            proj_T(swq, xT, bq, qt)
            swk = load_w(wnames["self_wk"], l, NDT, D, "wqkv")
            bk = load_bias_cols(bk_s_d, l, NDT)
            kt_sb = apool.tile([P, NDT, S], BF16, tag="kt")
            proj_T(swk, xT, bk, kt_sb, engine="s")
            swv = load_w(wnames["self_wv"], l, NDT, D, "wqkv")
            bv_bc = load_bcast(bv_s_d, l) if bv_on else None
            vg = apool.tile([P, NST, H * (DEP + 1)], BF16, tag="vaug")
            proj_V(swv, xT, bv_bc, vg, NST, engine="s")
            if l == 0:
                # Encoder output, transposed once (shared across layers).
                # Emitted after layer-0 QKV so the startup critical path
                # (embed -> xT -> q/k/v) owns the PE first; first needed
                # below for the cross-attn K projection.
                enc_bf = apool.tile([P, NST, D], BF16, tag="y")
                for st in range(NST):
                    et = gpool.tile([P, D], F32, tag="emb", bufs=4)
                    nc.sync.dma_start(out=et[:], in_=enc_d[st * P:(st + 1) * P, :])
                    nc.vector.tensor_copy(enc_bf[:, st, :], et[:])
                transpose_into(enc_bf, encT)
            ot_sb = apool.tile([P, NDT, S], BF16, tag="ot")
            attention(qt, kt_sb, vg, ot_sb, self_causal, False)
            # Cross-attn K/V depend only on encT; emitted right after the
            # self-attention body so their matmuls fill the PE bubble while
            # the softmax-normalize tail and LN1 chain drain.
            ktc = cross_k(l)
            vgc = cross_v(l)
            swo = load_w(wnames["self_wo"], l, NDT, D, "wqkv")
            bo_bc = load_bcast(bo_s_d, l) if bo_on else None
            y1 = apool.tile([P, NST, D], BF16, tag="y")
            proj_O(swo, ot_sb, x, y1, bo_bc)
            out1 = rpool.tile([P, NST, D], BF16, tag="res")
            g_bc = load_bcast(ln_d["g1"], l) if lnaff_on else None
            b_bc = load_bcast(ln_d["b1"], l) if lnaff_on else None
            layer_norm(y1, out1, g_bc, b_bc)

            # ---- cross attention ----
            o1T = apool.tile([P, NDT, S], BF16, tag="rT")
            transpose_into(out1, o1T)
            cwq = load_w(wnames["cross_wq"], l, NDT, D, "wqkv")
            bqc = load_bias_cols(bq_c_d, l, NDT)
            qt2 = apool.tile([P, NDT, S], BF16, tag="qt")
            proj_T(cwq, o1T, bqc, qt2)
            ot2 = apool.tile([P, NDT, S], BF16, tag="ot")
            attention(qt2, ktc, vgc, ot2, False, pm_on)
            cwo = load_w(wnames["cross_wo"], l, NDT, D, "wqkv")
            boc_bc = load_bcast(bo_c_d, l) if bo_on else None
            y2 = apool.tile([P, NST, D], BF16, tag="y")
            proj_O(cwo, ot2, out1, y2, boc_bc)
            out2 = rpool.tile([P, NST, D], BF16, tag="res")
            g_bc = load_bcast(ln_d["g2"], l) if lnaff_on else None
            b_bc = load_bcast(ln_d["b2"], l) if lnaff_on else None
            layer_norm(y2, out2, g_bc, b_bc)

            # ---- FFN ----
            o2T = apool.tile([P, NDT, S], BF16, tag="rT")
            transpose_into(out2, o2T)
            w1s = load_w(w1_d, l, NDT, F, "w1", pool=wbig)
            b1 = load_bias_cols(b1_d, l, NFT) if b1_on else None
            h_sb = hpool.tile([P, NFT, S], BF16, tag="hsb")
            for fm in range(NFT // 2):
                ps = pp.tile([P, 2, S], F32, tag="big")
                for f2 in range(2):
                    ft = fm * 2 + f2
                    for dt in range(NDT):
                        nc.tensor.matmul(
                            ps[:, f2, :],
                            lhsT=w1s[:, dt, ft * P:(ft + 1) * P],
                            rhs=o2T[:, dt, :],
                            start=(dt == 0),
                            stop=(dt == NDT - 1),
                        )
                if b1 is None:
                    nc.vector.tensor_scalar(
                        h_sb[:, fm * 2:fm * 2 + 2, :], ps[:], 0.0, None, OP.max
                    )
                else:
                    for f2 in range(2):
                        ft = fm * 2 + f2
                        nc.vector.tensor_scalar(
                            h_sb[:, ft, :], ps[:, f2, :],
                            b1[:, ft:ft + 1], 0.0, OP.add, OP.max,
                        )
            w2s = load_w(w2_d, l, NFT, D, "w2", pool=wbig)
            b2_bc = load_bcast(b2_d, l) if b2_on else None
            y3 = apool.tile([P, NST, D], BF16, tag="y")
            for sm in range(NST):
                ps2 = pp.tile([P, D], F32, tag="proj")
                for ft in range(NFT):
                    nc.tensor.matmul(
                        ps2[:],
                        lhsT=h_sb[:, ft, sm * P:(sm + 1) * P],
                        rhs=w2s[:, ft, :],
                        start=(ft == 0),
                        stop=(ft == NFT - 1),
                    )
                if b2_bc is not None:
                    nc.vector.tensor_tensor(ps2[:], ps2[:], b2_bc[:], OP.add)
                nc.vector.tensor_tensor(y3[:, sm, :], ps2[:], out2[:, sm, :], OP.add)
            g_bc = load_bcast(ln_d["g3"], l) if lnaff_on else None
            b_bc = load_bcast(ln_d["b3"], l) if lnaff_on else None
            if l == L - 1:
                layer_norm_final(y3, g_bc, b_bc)
            else:
                x = rpool.tile([P, NST, D], BF16, tag="res")
                layer_norm(y3, x, g_bc, b_bc)

    # All ScalarE functions we use (Exp, Ln, Identity, Relu, Copy) coexist in
    # the "natural_log_exp_and_others" table set.  Left to itself, the
    # table-load pass resolves Exp and Ln to *different* sets and thrashes
    # ACT_TABLE_LOADs (~1.3us each, hundreds of times).  Strip our functions
    # from every other set (keeping set order/ids intact for walrus) so all
    # activations resolve to the one shared set -> a single table load.
    _keep = "natural_log_exp_and_others"
    _mine = {AF.Exp, AF.Ln, AF.Identity, AF.Relu, AF.Copy}
    _orig = bacc.get_activation_tables

    def _pinned_tables(arch):
        tabs = _orig(arch)
        return {
            name: (fns if name == _keep else fns - _mine)
            for name, fns in tabs.items()
        }

    bacc.get_activation_tables = _pinned_tables
    try:
        nc.finalize()
    finally:
        bacc.get_activation_tables = _orig
    return nc


_PROGRAM_CACHE = {}


def _get_program(flags):
    if flags not in _PROGRAM_CACHE:
        _PROGRAM_CACHE[flags] = _build_program(flags)
    return _PROGRAM_CACHE[flags]


def _positional_encoding(max_pos, d_model):
    pos = np.arange(max_pos, dtype=np.float32)[:, None]
    i = np.arange(d_model, dtype=np.float32)[None, :]
    angle = pos / np.power(10000.0, 2.0 * np.floor(i / 2.0) / d_model)
    pe = np.where(
        np.arange(d_model)[None, :] % 2 == 0,
        np.sin(angle, dtype=np.float32),
        np.cos(angle, dtype=np.float32),
    )
    return pe.astype(np.float32)


def _classify_mask(la):
    la2 = np.asarray(la).reshape(S, S)
    if not la2.any():
        return MASK_ZERO
    causal = np.triu(np.ones((S, S), np.float32), 1)
    if np.array_equal(la2.astype(np.float32), causal):
        return MASK_CAUSAL
    return MASK_GENERIC


def kernel(**inputs):
    f32 = np.float32
    tokens = np.asarray(inputs["tokens"]).astype(np.int32)
    enc = np.asarray(inputs["enc_output"]).astype(f32)
    la = np.asarray(inputs["look_ahead_mask"]).astype(f32)
    pad = np.asarray(inputs["padding_mask"]).astype(f32)
    emb = np.asarray(inputs["emb"]).astype(f32)

    mask_mode = _classify_mask(la)
    pm_on = bool(pad.any())
    bv_on = bool(
        np.asarray(inputs["self_bv"]).any() or np.asarray(inputs["cross_bv"]).any()
    )
    bo_on = bool(
        np.asarray(inputs["self_bo"]).any() or np.asarray(inputs["cross_bo"]).any()
    )
    b2_on = bool(np.asarray(inputs["ffn_b2"]).any())
    b1_on = bool(np.asarray(inputs["ffn_b1"]).any())
    lnaff_on = any(
        not np.allclose(np.asarray(inputs[f"ln{i}_g"]), 1.0)
        or np.asarray(inputs[f"ln{i}_b"]).any()
        for i in (1, 2, 3)
    )
    flags = (mask_mode, pm_on, bv_on, bo_on, b2_on, lnaff_on, b1_on)
    nc = _get_program(flags)

    bf16 = ml_dtypes.bfloat16
    common = {
        "emb": emb,
        "pe": _positional_encoding(MAXPOS, D)[:S],
        "ffn_w1": np.asarray(inputs["ffn_w1"]).astype(bf16),
        "ffn_w2": np.asarray(inputs["ffn_w2"]).astype(bf16),
        "self_bq": np.asarray(inputs["self_bq"]).astype(f32),
        "self_bk": np.asarray(inputs["self_bk"]).astype(f32),
        "cross_bq": np.asarray(inputs["cross_bq"]).astype(f32),
        "cross_bk": np.asarray(inputs["cross_bk"]).astype(f32),
    }
    if b1_on:
        common["ffn_b1"] = np.asarray(inputs["ffn_b1"]).astype(f32)
    for prefix in ("self", "cross"):
        for nm in ("wq", "wk", "wv", "wo"):
            common[f"{prefix}_{nm}"] = np.asarray(inputs[f"{prefix}_{nm}"]).astype(bf16)
    if bv_on:
        common["self_bv"] = np.asarray(inputs["self_bv"]).astype(f32)
        common["cross_bv"] = np.asarray(inputs["cross_bv"]).astype(f32)
    if bo_on:
        common["self_bo"] = np.asarray(inputs["self_bo"]).astype(f32)
        common["cross_bo"] = np.asarray(inputs["cross_bo"]).astype(f32)
    if b2_on:
        common["ffn_b2"] = np.asarray(inputs["ffn_b2"]).astype(f32)
    if lnaff_on:
        for i in (1, 2, 3):
            common[f"ln{i}_g"] = np.asarray(inputs[f"ln{i}_g"]).astype(f32)
            common[f"ln{i}_b"] = np.asarray(inputs[f"ln{i}_b"]).astype(f32)
    if mask_mode == MASK_GENERIC:
        common["maskT"] = np.ascontiguousarray(
            la.reshape(S, S).T * np.float32(-1e9)
        ).astype(f32)

    in_maps = []
    for b in range(B):
        m = dict(common)
        m["tokens"] = np.ascontiguousarray(tokens[b])
        m["enc"] = np.ascontiguousarray(enc[b])
        if pm_on:
            m["pm"] = np.ascontiguousarray(pad[b, 0, 0] * np.float32(-1e9))
        in_maps.append(m)

    global _last_flags, _last_in_maps
    _last_flags = flags
    _last_in_maps = in_maps

    res = run_bass_kernel_spmd(nc, in_maps, list(range(B)))
    out = np.stack([res.results[i]["out"] for i in range(B)], axis=0)
    return out.astype(np.float32)


_last_flags = None
_last_in_maps = None

